# revision 13
# baseline (speedup 1.0000x reference)
"""Trainium2 Bass kernel for nn_DiscreteAutoregressiveFlow (sampling, forward).

Math: `inputs` is an exact one-hot [B, L, V] tensor. For a row holding token v
the reference reduces to out_row = one_hot((scale_tok[v]*v + loc_tok[v]) % V),
or the zero row when scale_tok[v] == 0, where loc_tok/scale_tok are argmaxes
of W[v]+b halves (host-precomputable from W/b alone). So the whole flow is a
fixed linear map applied per row, which TensorE evaluates as a matmul against
a host-built table. All products/sums are exact in fp8/f32/bf16.

Device pipeline (memory-bound streaming):
  - Host packs TWO one-hot rows per fp8 input element: xt[v, c] = [rowA==v]
    + 2*[rowB==v] (values {0,1,2,3}), with the vocab dim on partitions
    (64*half + v) -> [128, 4096] fp8 per core = 525 KB input.
  - The stationary table emits the output one-hot as EVEN-BIT powers:
    T[v, u//4] = 4^(u%4) for u = cmap[v] (values {1,4,16,64}); rowB's factor
    2 shifts its contribution to the odd bits for free, so each PSUM value
    is a sum of at most two DISTINCT powers of two (<= 192), exact in bf16.
    Each matmul emits [32, 512] (16 data lanes x 2 halves + zero pad);
    4 matmuls pack one PSUM bank at tile_position col offsets 0/32/64/96 so
    ONE cast-copy drains four matmuls. 8 matmuls, 2 banks, 2 copies, 256 KB
    bf16 output. Host decodes with bit masks (A = v & 0x55 even bits,
    B = (v >> 1) & 0x55).

Scheduling: each HWDGE dma_start costs ~650ns of SERIAL sequencer time
(DIRECT2D descriptor generation) and completion semaphores fire ~2us after
the last byte, so DMAs are few, tapered, and split across both HWDGE rings
(SP + ACT); each bank's output DMA is dispatched from the engine that ran
its copy. Sharding: pure data parallel over B*L rows, 8 cores.
"""

import numpy as np

V = 64
P = 128
N_CORES = 8
B, L = 16, 8192
ROWS = B * L                      # 131072
ROWS_PER_CORE = ROWS // N_CORES   # 16384
HALF = ROWS_PER_CORE // 2         # 8192 rows per half
COLS = HALF // 2                  # 4096 columns (2 rows per fp8 element)
MM_N = 512                        # PSUM bank = 512 f32
N_WIN = COLS // MM_N              # 8 matmul windows
N_BANKS = N_WIN // 4              # 4 windows packed per PSUM bank
OUT_W = N_BANKS * MM_N            # 1024 output columns (bit-packed bf16)

# Input chunks (width, ring), tapered and interleaved across both HWDGE
# rings: a small head chunk so TensorE starts at the first completion
# receipt, a small tail chunk so the last matmuls wait minimally.
IN_CHUNKS = ((512, "sp"), (1024, "act"), (1536, "sp"), (1024, "act"))
assert sum(w for w, _ in IN_CHUNKS) == COLS
assert all(w % MM_N == 0 for w, _ in IN_CHUNKS)

_CACHE = {}


def _build_nc(in_chunks=IN_CHUNKS):
    import concourse.bacc as bacc
    import concourse.mybir as mybir
    from concourse.tile import TileContext

    f32 = mybir.dt.float32
    bf16 = mybir.dt.bfloat16
    fp8 = mybir.dt.float8e4

    nc = bacc.Bacc("TRN2", target_bir_lowering=False, name="daf_mm")
    xt = nc.dram_tensor("xt", [P, COLS], fp8, kind="ExternalInput")
    mt = nc.dram_tensor("mt", [P, 32], fp8, kind="ExternalInput")
    yt = nc.dram_tensor("yt", [P, OUT_W], bf16, kind="ExternalOutput")

    with TileContext(nc) as tc:
        with (
            tc.tile_pool(name="const", bufs=1) as constp,
            tc.tile_pool(name="io", bufs=1) as iop,
            tc.tile_pool(name="ps", bufs=1, space="PSUM") as psp,
        ):
            # Stationary nibble-lane table: tiny, first dispatch on SP so
            # both ACT input chunks move up one serial-dispatch slot (the
            # last ACT chunk's completion receipt gates the tail).
            mt_st = constp.tile([P, 32], fp8, tag="mt")
            nc.sync.dma_start(mt_st[:], mt[:])

            # All input DMAs dispatched next, alternating rings.
            in_tiles = []   # (tile, col_start, width)
            cs = 0
            for ci, (cw, q) in enumerate(in_chunks):
                xtile = iop.tile([P, cw], fp8, tag=f"x{ci}")
                eng = nc.sync if q == "sp" else nc.scalar
                eng.dma_start(xtile[:], xt[:][:, cs : cs + cw])
                in_tiles.append((xtile, cs, cw))
                cs += cw

            def rhs_window(w):
                col = w * MM_N
                for xtile, xcs, xcw in in_tiles:
                    if xcs <= col and col + MM_N <= xcs + xcw:
                        return xtile[:][:, col - xcs : col - xcs + MM_N]
                raise AssertionError(w)

            ps_tiles = [
                psp.tile([P, MM_N], f32, tag=f"ps{b}", name=f"ps{b}")
                for b in range(N_BANKS)
            ]
            o0 = iop.tile([P, MM_N], bf16, tag="o0")
            o1 = iop.tile([P, MM_N], bf16, tag="o1")
            for w in range(N_WIN):
                b, s = w // 4, w % 4
                ps = ps_tiles[b]
                nc.tensor.matmul(
                    ps[:][32 * s : 32 * s + 32, :],
                    mt_st[:],
                    rhs_window(w),
                    # Every matmul is its own "group": start=True clears the
                    # bank's has_written bits (data of other strips persists)
                    # and overwrites this strip; start=False would accumulate
                    # onto stale PSUM state from a previous NEFF execution.
                    start=True,
                    stop=True,
                    tile_position=(0, 32 * s),
                    skip_group_check=True,
                )
                if s == 3:
                    # Bank 0 -> DVE copy, output DMA from SP; bank 1 -> ACT
                    # copy with its output DMA dispatched from ACT itself
                    # (no cross-engine hop on the last link).
                    if b == 0:
                        nc.vector.tensor_copy(o0[:], ps[:])
                        nc.sync.dma_start(yt[:][:, :MM_N], o0[:])
                    else:
                        nc.scalar.copy(o1[:], ps[:])
                        nc.scalar.dma_start(yt[:][:, MM_N:], o1[:])

    nc.finalize()
    return nc


def _get_nc(in_chunks=IN_CHUNKS):
    key = in_chunks
    if key not in _CACHE:
        _CACHE[key] = _build_nc(in_chunks)
    return _CACHE[key]


def _host_mtab(W: np.ndarray, b: np.ndarray) -> np.ndarray:
    """[128, 32] fp8 nibble-lane table: T[v, u//4] = 4^(u%4), u = cmap[v];
    lanes 0-15 serve half A (partitions 0-63), lanes 16-31 half B."""
    import ml_dtypes

    net = W.astype(np.float32) + b.astype(np.float32)[None, :]   # [V, 2V]
    loc_tok = np.argmax(net[:, :V], axis=1)
    scale_tok = np.argmax(net[:, V:], axis=1)
    tgt = (scale_tok * np.arange(V, dtype=np.int64) + loc_tok) % V
    t = np.zeros((V, 16), dtype=np.float32)
    nz = np.flatnonzero(scale_tok != 0)
    t[nz, tgt[nz] // 4] = (4.0 ** (tgt[nz] % 4)).astype(np.float32)
    mt = np.zeros((P, 32), dtype=np.float32)
    mt[:V, :16] = t
    mt[V:, 16:] = t
    return mt.astype(ml_dtypes.float8_e4m3)


def _host_in_maps(inputs: np.ndarray, W: np.ndarray, b: np.ndarray):
    import ml_dtypes

    x = inputs.reshape(N_CORES, 2, HALF, V)
    # [core, half, row, v] -> [core, half, v, row], then pack row pairs:
    # even row -> +1, odd row -> +2 (values {0,1,2,3}, exact in fp8).
    xt = np.ascontiguousarray(x.transpose(0, 1, 3, 2))
    packed = xt[..., 0::2] + 2.0 * xt[..., 1::2]          # [core, 2, V, COLS]
    xt4 = packed.reshape(N_CORES, P, COLS).astype(ml_dtypes.float8_e4m3)
    mt = _host_mtab(W, b)
    return [{"xt": xt4[c], "mt": mt} for c in range(N_CORES)]


def _host_gather(results, shape, dtype) -> np.ndarray:
    yt = np.stack([np.asarray(r["yt"]) for r in results])   # [8, 128, 1024] bf16
    ints = yt.astype(np.float32).astype(np.int32)           # values <= 192 exact
    v6 = ints.reshape(N_CORES, 4, 2, 16, N_BANKS, MM_N)     # [c, s, h, j, b, nl]
    i4 = np.arange(4)
    a_bits = (v6[..., None] >> (2 * i4)) & 1                # even bits: row A
    b_bits = (v6[..., None] >> (2 * i4 + 1)) & 1            # odd bits: row B
    both = np.stack([a_bits, b_bits], axis=-2)              # [c,s,h,j,b,nl,ab,i]
    arr = both.transpose(0, 2, 4, 1, 5, 6, 3, 7)            # [c,h,b,s,nl,ab,j,i]
    y = arr.reshape(N_CORES, 2, HALF, V).reshape(ROWS, V).astype(np.float32)
    return y.reshape(shape).astype(dtype, copy=False)


def kernel(inputs: np.ndarray, W: np.ndarray, b: np.ndarray) -> np.ndarray:
    from concourse import bass_utils

    in_maps = _host_in_maps(np.asarray(inputs), np.asarray(W), np.asarray(b))
    nc = _get_nc()
    res = bass_utils.run_bass_kernel_spmd(nc, in_maps, core_ids=list(range(N_CORES)))
    return _host_gather(res.results, inputs.shape, inputs.dtype)
